# revision 21
# baseline (speedup 1.0000x reference)
"""minGRU cell kernel for 8 Trainium2 NeuronCores.

Math (per batch b, all in linear domain — the recurrence is a convex
combination of positive values, so no log-space is needed):
    gh[s, :] = x[s, :] @ W.T + b          # (S, 2H)
    gate, hidden = gh[:, :H], gh[:, H:]
    z = sigmoid(gate);  a = 1 - z = sigmoid(-gate)
    g(hidden) = relu(hidden) + min(sigmoid(hidden), 0.5)
    h_t = a_t * h_{t-1} + z_t * g_t       # scan over s

Distribution: pure data parallel over B (8 batches -> 8 cores).

Device layout: channels on SBUF partitions, time on the free dim:
    out[o, s] = sum_i WT[i, o] * xT[i, s]
so the matmul result lands directly in the layout the DVE
tensor_tensor_scan instruction needs.  Matmuls run as float32r
(full-rate fp32 PE mode).

Perf structure (vs the naive version):
  * W is packed host-side per output-tile (o-major), so the first
    j-chain only needs 1 MB of W + 2.1 MB of x before the PE can
    start — the old k-major layout needed all 8.4 MB of W first
    (26 us of PE idle at startup).
  * DMA descriptor issue is spread across three queues (W on the
    Vector queue, x on Sync, consts + output stores on GpSimd) so
    descriptor serialization never gates the startup transfers.
  * Within each (chunk, j) iteration the hidden chain runs BEFORE the
    gate chain: the post-matmul serial tail (sigmoid/relu/min-add on
    the hidden projection) overlaps the gate matmuls, and the final
    chunk's post-ops are split into 256-column halves, shortening the
    end-of-kernel drain.
"""

from contextlib import ExitStack

import numpy as np

import concourse.bass as bass
import concourse.bacc as bacc
import concourse.mybir as mybir
import concourse.tile as tile
from concourse.bass_utils import run_bass_kernel_spmd

B, S, DIN, DH = 8, 4096, 1024, 1024
CH = 512                 # time-chunk (free dim of each matmul / scan)
NCHUNK = S // CH         # 8
KT = DIN // 128          # 8 contraction tiles
JT = DH // 128           # 8 channel tiles (per gate/hidden half)

F32 = mybir.dt.float32
F16 = mybir.dt.float16
AF = mybir.ActivationFunctionType
OP = mybir.AluOpType

_prog_cache = {}


def _build_program() -> bass.Bass:
    nc = bacc.Bacc("TRN2", target_bir_lowering=False, debug=False,
                   num_devices=B)
    xt = nc.dram_tensor("xt", (KT, 128, S), F16, kind="ExternalInput")
    # per-o packed weights: wp[o*128+p, k*128+c] = W[o*128+c, k*128+p]
    wp = nc.dram_tensor("wp", (2 * DH, DIN), F16, kind="ExternalInput")
    bias = nc.dram_tensor("bias", (128, 2 * JT), F32, kind="ExternalInput")
    nbias = nc.dram_tensor("nbias", (128, 2 * JT), F32, kind="ExternalInput")
    h0 = nc.dram_tensor("h0", (128, JT), F32, kind="ExternalInput")
    out = nc.dram_tensor("out", (DH, S), F32, kind="ExternalOutput")

    with ExitStack() as ctx:
        tc = ctx.enter_context(tile.TileContext(nc))
        cpool = ctx.enter_context(tc.tile_pool(name="const", bufs=1))
        wpool = ctx.enter_context(tc.tile_pool(name="w", bufs=1))
        xpool = ctx.enter_context(tc.tile_pool(name="x", bufs=2))
        spool = ctx.enter_context(tc.tile_pool(name="tmp", bufs=2))
        abpool = ctx.enter_context(tc.tile_pool(name="ab", bufs=3))
        hpool = ctx.enter_context(tc.tile_pool(name="h", bufs=2))
        # 2 tiles per chain, bufs=4 => exactly 2 chains in flight.  More
        # (bufs=8) lets the PE run 4 chains ahead of ACT, and the
        # then-permanent concurrent ACT psum reads slow every matmul's
        # accumulation by ~20% (measured 227 -> 272 ns cadence).
        ppool = ctx.enter_context(tc.tile_pool(name="psum", bufs=4, space="PSUM"))

        # Consts go over the (otherwise idle) GpSimd queue, then are
        # re-materialized on the engines that consume them (ACT for
        # bias/nbias, DVE for h0) so hot-loop instructions never carry
        # a DMA sync-wait.
        bias_d = cpool.tile([128, 2 * JT], F32, tag="bias_d")
        nc.gpsimd.dma_start(bias_d[:], bias[:, :])
        nbias_d = cpool.tile([128, 2 * JT], F32, tag="nbias_d")
        nc.gpsimd.dma_start(nbias_d[:], nbias[:, :])
        h0_d = cpool.tile([128, JT], F32, tag="h0_d")
        nc.gpsimd.dma_start(h0_d[:], h0[:, :])

        # Startup is DMA-bandwidth-bound: ~12.6 MB (W + x chunks 0/1)
        # must land in the first ~40 us.  Startup-critical input
        # transfers are sequenced in exact need order so later
        # transfers never steal bandwidth from earlier ones:
        #   w(j=0), x chunk 0, w(j=1..5), x chunk 1, w(j=6,7)
        # The first batch is split across the Sync AND Scalar queues
        # (two descriptors in flight ramps the DMA engines up faster);
        # everything after runs on Sync alone so the Scalar queue is
        # free for the ACT hot loop by the time psum drains start.
        def w_load(o, queue=nc.sync, per_k=False):
            w_t = wpool.tile([128, DIN], F16, tag=f"w{o}")
            if per_k:
                # per-k-slice descriptors: the first matmul only waits
                # for the first 64 KB, not the whole 512 KB tile.
                for k in range(KT):
                    queue.dma_start(w_t[:, k * 128:(k + 1) * 128],
                                    wp[o * 128:(o + 1) * 128,
                                       k * 128:(k + 1) * 128])
            else:
                queue.dma_start(w_t[:], wp[o * 128:(o + 1) * 128, :])
            wts[o] = w_t

        def x_load(c, alternate=False):
            # per-k descriptors: chunk-0 chains consume tiles
            # k-progressively as they land.
            s0 = c * CH
            xts = []
            for k in range(KT):
                x_t = xpool.tile([128, CH], F16, tag=f"x{k}")
                q = nc.scalar if (alternate and k % 2 == 1) else nc.sync
                q.dma_start(x_t[:], xt[k, :, s0:s0 + CH])
                xts.append(x_t)
            xts_by_chunk[c] = [
                (lambda t: (lambda lo, hi: t[:, lo:hi]))(x_t) for x_t in xts]

        def x_load_big(c):
            # steady-state chunks: ONE descriptor for the whole chunk
            # (8 strided k-blocks) => one DMA semaphore instead of 8 on
            # the Tensor queue.
            s0 = c * CH
            xbig = xpool.tile([128, KT * CH], F16, tag="xbig")
            nc.sync.dma_start(xbig[:].rearrange("p (k t) -> p k t", k=KT),
                              xt[:, :, s0:s0 + CH].rearrange("k p t -> p k t"))
            xts_by_chunk[c] = [
                (lambda kk: (lambda lo, hi:
                             xbig[:, kk * CH + lo:kk * CH + hi]))(k)
                for k in range(KT)]

        wts = [None] * (2 * JT)
        xts_by_chunk = {}
        # Startup is DMA-bandwidth-bound; sequence the first tiles in
        # per-matmul need order across three queues so the first matmul
        # only waits for w(JT+0) || x0.k0 (~1.4 us) instead of their sum:
        #   Sync:   w(JT+0), x0.k2/k4/k6, w pairs j=1..5, x chunk 1, j=6,7
        #   Scalar: x0.k0, x0.k1/k3/k5/k7
        #   GpSimd: (consts,) w(0)
        # (Splitting W into per-k slices was tried and regresses: 256 B
        # contiguous runs collapse DMA efficiency.)
        w_load(0, nc.gpsimd)
        w_load(JT + 0)
        x0_ts = []
        for k in range(KT):
            x_t = xpool.tile([128, CH], F16, tag=f"x{k}", name=f"x0_{k}")
            q = nc.scalar if (k == 0 or k % 2 == 1) else nc.sync
            q.dma_start(x_t[:], xt[k, :, 0:CH])
            x0_ts.append(x_t)
        xts_by_chunk[0] = [
            (lambda t: (lambda lo, hi: t[:, lo:hi]))(x_t) for x_t in x0_ts]
        for j in range(1, 6):
            w_load(JT + j), w_load(j)
        x_load(1)
        for j in range(6, 8):
            w_load(JT + j), w_load(j)

        # Const copies: first ACT op needs bias_t at ~17 us.
        bias_t = cpool.tile([128, 2 * JT], F32, tag="bias")
        nc.scalar.copy(bias_t[:], bias_d[:])
        nbias_t = cpool.tile([128, 2 * JT], F32, tag="nbias")
        nc.scalar.copy(nbias_t[:], nbias_d[:])
        h0_t = cpool.tile([128, JT], F32, tag="h0")
        nc.vector.tensor_copy(h0_t[:], h0_d[:])

        prev_h = [None] * JT
        g7pool = ctx.enter_context(tc.tile_pool(name="g7", bufs=1))

        def hidden_part(c, j, sub, pool=None):
            """Hidden-half matmuls + post-ops (sg/relu/g) for chain (c,j).
            Returns a g-lookup closure.  `pool` pins the g tile in a
            dedicated pool so it can outlive the spool rotation (used to
            hoist the last chain's hidden work ahead of the tail)."""
            xl = xts_by_chunk[c]
            ph = ppool.tile([128, CH], F32, tag="psum", name="ph")
            for k in range(KT):
                nc.tensor.matmul(
                    ph[:],
                    lhsT=wts[JT + j][:, k * 128:(k + 1) * 128],
                    rhs=xl[k](0, CH),
                    start=(k == 0),
                    stop=(k == KT - 1),
                )
            tiles = []
            for f0 in range(0, CH, sub):
                fs = slice(f0, f0 + sub)
                sg_t = spool.tile([128, sub], F32, tag="sg")
                nc.scalar.activation(sg_t[:], ph[:, fs], AF.Sigmoid,
                                     bias=bias_t[:, JT + j:JT + j + 1],
                                     scale=1.0)
                r_t = spool.tile([128, sub], F32, tag="r")
                nc.scalar.activation(r_t[:], ph[:, fs], AF.Relu,
                                     bias=bias_t[:, JT + j:JT + j + 1],
                                     scale=1.0)
                # g = min(sigmoid(hidden), 0.5) + relu(hidden)
                if pool is None:
                    g_t = spool.tile([128, sub], F32, tag="g")
                else:
                    g_t = pool.tile([128, sub], F32, tag=f"g7_{f0}")
                nc.vector.scalar_tensor_tensor(g_t[:], sg_t[:], 0.5,
                                               r_t[:], op0=OP.min,
                                               op1=OP.add)
                tiles.append((f0, sub, g_t))

            def g_at(f0, w):
                for t0, tw, t in tiles:
                    if t0 <= f0 and f0 + w <= t0 + tw:
                        return t[:, f0 - t0:f0 - t0 + w]
                raise KeyError((f0, w))
            return g_at

        def gate_part(c, j, sub, g_at, gate_split=1, a_on_dve=False):
            """Gate-half matmuls + a/z/b/scan/store for chain (c,j).
            `gate_split` > 1 runs the gate matmuls as that many column
            sub-chains so the earlier blocks' post-ops overlap the later
            blocks' matmuls, and `a_on_dve` computes a = 1-z on the DVE
            instead of a second ACT sigmoid (both shorten the serial
            post-matmul tail; used for the very last chain only)."""
            xl = xts_by_chunk[c]
            s0 = c * CH
            pg = ppool.tile([128, CH], F32, tag="psum", name="pg")
            gw = CH // gate_split
            for lo in range(0, CH, gw):
                for k in range(KT):
                    nc.tensor.matmul(
                        pg[:, lo:lo + gw],
                        lhsT=wts[j][:, k * 128:(k + 1) * 128],
                        rhs=xl[k](lo, lo + gw),
                        start=(k == 0),
                        stop=(k == KT - 1),
                    )
            h_t = hpool.tile([128, CH], F32, tag=f"h{j}")
            for f0 in range(0, CH, sub):
                fs = slice(f0, f0 + sub)
                z_t = spool.tile([128, sub], F32, tag="z")
                nc.scalar.activation(z_t[:], pg[:, fs], AF.Sigmoid,
                                     bias=bias_t[:, j:j + 1], scale=1.0)
                a_t = abpool.tile([128, sub], F32, tag="a")
                if a_on_dve:
                    nc.vector.tensor_scalar(a_t[:], z_t[:], -1.0, 1.0,
                                            op0=OP.mult, op1=OP.add)
                else:
                    nc.scalar.activation(a_t[:], pg[:, fs], AF.Sigmoid,
                                         bias=nbias_t[:, j:j + 1], scale=-1.0)
                b_t = abpool.tile([128, sub], F32, tag="b")
                nc.vector.tensor_mul(b_t[:], z_t[:], g_at(f0, sub))
                # ---- scan: h = a*h_prev + b along time
                if f0 == 0:
                    init = (h0_t[:, j:j + 1] if c == 0
                            else prev_h[j][:, CH - 1:CH])
                else:
                    init = h_t[:, f0 - 1:f0]
                nc.vector.tensor_tensor_scan(h_t[:, fs], a_t[:], b_t[:],
                                             init, op0=OP.mult, op1=OP.add)
            prev_h[j] = h_t
            # GpSimd's end-of-kernel DRAIN detects DMA completion
            # slowly (~6 us); keep the final chunks' stores on Sync
            # (idle by then) so the kernel end isn't gated on it.
            # The final chunk stores per-sub so the last transfer
            # is short.
            out_q = nc.gpsimd if c < NCHUNK - 2 else nc.sync
            if c == NCHUNK - 1:
                for f0 in range(0, CH, sub):
                    out_q.dma_start(
                        out[j * 128:(j + 1) * 128, s0 + f0:s0 + f0 + sub],
                        h_t[:, f0:f0 + sub])
            else:
                out_q.dma_start(out[j * 128:(j + 1) * 128, s0:s0 + CH],
                                h_t[:])

        def chain(c, j, sub, gate_split=1, a_on_dve=False):
            gate_part(c, j, sub, hidden_part(c, j, sub), gate_split,
                      a_on_dve)

        # Chunks 0/1: single-chunk chains, interleaved so chains line
        # up with the W/x arrival schedule.
        for c, j in ([(0, j) for j in range(6)] + [(1, j) for j in range(4)]
                     + [(0, 6), (0, 7)] + [(1, j) for j in range(4, 8)]):
            chain(c, j, CH)
        # Chunks 2..7 streamed singly with batched x transfers.  The last
        # chain's hidden half is hoisted to the top of the final chunk so
        # the end-of-kernel tail is only its gate post-ops; those run at
        # 128 wide over two gate matmul half-chains so the serial
        # a/z/mul/scan/store tail after the last matmul is short.
        for c in range(2, NCHUNK):
            x_load_big(c)
            if c == NCHUNK - 1:
                for j in range(JT - 1):
                    chain(c, j, CH if j < JT - 2 else CH // 2)
                chain(c, JT - 1, CH // 2, gate_split=2, a_on_dve=True)
            else:
                for j in range(JT):
                    chain(c, j, CH)

    nc.compile()
    return nc


def _run(inputs, trace=False, **spmd_kwargs):
    x = np.asarray(inputs["x"], dtype=np.float32)
    h = np.asarray(inputs["h"], dtype=np.float32)
    W = np.asarray(inputs["W"], dtype=np.float32)
    b = np.asarray(inputs["b"], dtype=np.float32)

    xt_all = np.ascontiguousarray(x.transpose(0, 2, 1)).astype(np.float16).reshape(
        B, KT, 128, S)                                             # (B,KT,128,S)
    # wp[o*128+p, k*128+c] = W[o*128+c, k*128+p]
    WP = np.ascontiguousarray(
        W.reshape(2 * JT, 128, KT, 128).transpose(0, 3, 2, 1)
        .reshape(2 * DH, DIN)).astype(np.float16)
    bias_t = np.ascontiguousarray(b.reshape(2 * JT, 128).T)        # (128, 2JT)
    nbias_t = np.ascontiguousarray(-bias_t)
    h0_all = np.ascontiguousarray(
        h[:, 0, :].reshape(B, JT, 128).transpose(0, 2, 1))         # (B, 128, JT)

    if "prog" not in _prog_cache:
        _prog_cache["prog"] = _build_program()
    nc = _prog_cache["prog"]

    in_maps = [
        {"xt": xt_all[c], "wp": WP, "bias": bias_t, "nbias": nbias_t,
         "h0": h0_all[c]}
        for c in range(B)
    ]
    res = run_bass_kernel_spmd(nc, in_maps, list(range(B)), trace=trace,
                               **spmd_kwargs)
    out = np.stack([res.results[c]["out"].T for c in range(B)], axis=0)
    return np.ascontiguousarray(out), res


def kernel(**inputs) -> np.ndarray:
    return _run(inputs)[0]



# revision 22
# speedup vs baseline: 1.0156x; 1.0156x over previous
"""minGRU cell kernel for 8 Trainium2 NeuronCores.

Math (per batch b, all in linear domain — the recurrence is a convex
combination of positive values, so no log-space is needed):
    gh[s, :] = x[s, :] @ W.T + b          # (S, 2H)
    gate, hidden = gh[:, :H], gh[:, H:]
    z = sigmoid(gate);  a = 1 - z = sigmoid(-gate)
    g(hidden) = relu(hidden) + min(sigmoid(hidden), 0.5)
    h_t = a_t * h_{t-1} + z_t * g_t       # scan over s

Distribution: pure data parallel over B (8 batches -> 8 cores).

Device layout: channels on SBUF partitions, time on the free dim:
    out[o, s] = sum_i WT[i, o] * xT[i, s]
so the matmul result lands directly in the layout the DVE
tensor_tensor_scan instruction needs.  Matmuls run as float32r
(full-rate fp32 PE mode).

Perf structure (vs the naive version):
  * W is packed host-side per output-tile (o-major), so the first
    j-chain only needs 1 MB of W + 2.1 MB of x before the PE can
    start — the old k-major layout needed all 8.4 MB of W first
    (26 us of PE idle at startup).
  * DMA descriptor issue is spread across three queues (W on the
    Vector queue, x on Sync, consts + output stores on GpSimd) so
    descriptor serialization never gates the startup transfers.
  * Within each (chunk, j) iteration the hidden chain runs BEFORE the
    gate chain: the post-matmul serial tail (sigmoid/relu/min-add on
    the hidden projection) overlaps the gate matmuls, and the final
    chunk's post-ops are split into 256-column halves, shortening the
    end-of-kernel drain.
"""

from contextlib import ExitStack

import numpy as np

import concourse.bass as bass
import concourse.bacc as bacc
import concourse.mybir as mybir
import concourse.tile as tile
from concourse.bass_utils import run_bass_kernel_spmd

B, S, DIN, DH = 8, 4096, 1024, 1024
CH = 512                 # time-chunk (free dim of each matmul / scan)
NCHUNK = S // CH         # 8
KT = DIN // 128          # 8 contraction tiles
JT = DH // 128           # 8 channel tiles (per gate/hidden half)

F32 = mybir.dt.float32
F16 = mybir.dt.float16
AF = mybir.ActivationFunctionType
OP = mybir.AluOpType

_prog_cache = {}


def _build_program() -> bass.Bass:
    nc = bacc.Bacc("TRN2", target_bir_lowering=False, debug=False,
                   num_devices=B)
    xt = nc.dram_tensor("xt", (KT, 128, S), F16, kind="ExternalInput")
    # per-o packed weights: wp[o*128+p, k*128+c] = W[o*128+c, k*128+p]
    wp = nc.dram_tensor("wp", (2 * DH, DIN), F16, kind="ExternalInput")
    bias = nc.dram_tensor("bias", (128, 2 * JT), F32, kind="ExternalInput")
    nbias = nc.dram_tensor("nbias", (128, 2 * JT), F32, kind="ExternalInput")
    h0 = nc.dram_tensor("h0", (128, JT), F32, kind="ExternalInput")
    out = nc.dram_tensor("out", (DH, S), F32, kind="ExternalOutput")

    with ExitStack() as ctx:
        tc = ctx.enter_context(tile.TileContext(nc))
        cpool = ctx.enter_context(tc.tile_pool(name="const", bufs=1))
        wpool = ctx.enter_context(tc.tile_pool(name="w", bufs=1))
        xpool = ctx.enter_context(tc.tile_pool(name="x", bufs=2))
        spool = ctx.enter_context(tc.tile_pool(name="tmp", bufs=2))
        abpool = ctx.enter_context(tc.tile_pool(name="ab", bufs=3))
        hpool = ctx.enter_context(tc.tile_pool(name="h", bufs=2))
        # 2 tiles per chain, bufs=4 => exactly 2 chains in flight.  More
        # (bufs=8) lets the PE run 4 chains ahead of ACT, and the
        # then-permanent concurrent ACT psum reads slow every matmul's
        # accumulation by ~20% (measured 227 -> 272 ns cadence).
        ppool = ctx.enter_context(tc.tile_pool(name="psum", bufs=4, space="PSUM"))

        # Consts go over the (otherwise idle) GpSimd queue, then are
        # re-materialized on the engines that consume them (ACT for
        # bias/nbias, DVE for h0) so hot-loop instructions never carry
        # a DMA sync-wait.
        bias_d = cpool.tile([128, 2 * JT], F32, tag="bias_d")
        nc.gpsimd.dma_start(bias_d[:], bias[:, :])
        nbias_d = cpool.tile([128, 2 * JT], F32, tag="nbias_d")
        nc.gpsimd.dma_start(nbias_d[:], nbias[:, :])
        h0_d = cpool.tile([128, JT], F32, tag="h0_d")
        nc.gpsimd.dma_start(h0_d[:], h0[:, :])

        # Startup is DMA-bandwidth-bound: ~12.6 MB (W + x chunks 0/1)
        # must land in the first ~40 us.  Startup-critical input
        # transfers are sequenced in exact need order so later
        # transfers never steal bandwidth from earlier ones:
        #   w(j=0), x chunk 0, w(j=1..5), x chunk 1, w(j=6,7)
        # The first batch is split across the Sync AND Scalar queues
        # (two descriptors in flight ramps the DMA engines up faster);
        # everything after runs on Sync alone so the Scalar queue is
        # free for the ACT hot loop by the time psum drains start.
        def w_load(o, queue=nc.sync, per_k=False):
            w_t = wpool.tile([128, DIN], F16, tag=f"w{o}")
            if per_k:
                # per-k-slice descriptors: the first matmul only waits
                # for the first 64 KB, not the whole 512 KB tile.
                for k in range(KT):
                    queue.dma_start(w_t[:, k * 128:(k + 1) * 128],
                                    wp[o * 128:(o + 1) * 128,
                                       k * 128:(k + 1) * 128])
            else:
                queue.dma_start(w_t[:], wp[o * 128:(o + 1) * 128, :])
            wts[o] = w_t

        def x_load(c, alternate=False):
            # per-k descriptors: chunk-0 chains consume tiles
            # k-progressively as they land.
            s0 = c * CH
            xts = []
            for k in range(KT):
                x_t = xpool.tile([128, CH], F16, tag=f"x{k}")
                q = nc.scalar if (alternate and k % 2 == 1) else nc.sync
                q.dma_start(x_t[:], xt[k, :, s0:s0 + CH])
                xts.append(x_t)
            xts_by_chunk[c] = [
                (lambda t: (lambda lo, hi: t[:, lo:hi]))(x_t) for x_t in xts]

        def x_load_big(c):
            # steady-state chunks: ONE descriptor for the whole chunk
            # (8 strided k-blocks) => one DMA semaphore instead of 8 on
            # the Tensor queue.
            s0 = c * CH
            xbig = xpool.tile([128, KT * CH], F16, tag="xbig")
            nc.sync.dma_start(xbig[:].rearrange("p (k t) -> p k t", k=KT),
                              xt[:, :, s0:s0 + CH].rearrange("k p t -> p k t"))
            xts_by_chunk[c] = [
                (lambda kk: (lambda lo, hi:
                             xbig[:, kk * CH + lo:kk * CH + hi]))(k)
                for k in range(KT)]

        wts = [None] * (2 * JT)
        xts_by_chunk = {}
        # Chunk-0's end is data-bound (w pair + ~1.3 MB) no matter how
        # early the first matmul fires, and the DMA queues ramp slowly in
        # the first few us, so keep the simple smooth split: w tiles whole
        # on Sync, x chunk 0 alternating between Sync and Scalar, gate
        # w(j=0) on Scalar.  (Tried and regressed: per-k W slices [256 B
        # runs collapse DMA efficiency], x0.k0-first-on-Scalar with w(0)
        # on GpSimd [Sync ramp starves the even x tiles].)
        w_load(JT + 0)
        w_load(0, nc.scalar)
        x_load(0, alternate=True)
        for j in range(1, 6):
            w_load(JT + j), w_load(j)
        x_load(1)
        for j in range(6, 8):
            w_load(JT + j), w_load(j)

        # Const copies: first ACT op needs bias_t at ~17 us.
        bias_t = cpool.tile([128, 2 * JT], F32, tag="bias")
        nc.scalar.copy(bias_t[:], bias_d[:])
        nbias_t = cpool.tile([128, 2 * JT], F32, tag="nbias")
        nc.scalar.copy(nbias_t[:], nbias_d[:])
        h0_t = cpool.tile([128, JT], F32, tag="h0")
        nc.vector.tensor_copy(h0_t[:], h0_d[:])

        prev_h = [None] * JT
        g7pool = ctx.enter_context(tc.tile_pool(name="g7", bufs=1))

        def hidden_part(c, j, sub, pool=None):
            """Hidden-half matmuls + post-ops (sg/relu/g) for chain (c,j).
            Returns a g-lookup closure.  `pool` pins the g tile in a
            dedicated pool so it can outlive the spool rotation (used to
            hoist the last chain's hidden work ahead of the tail)."""
            xl = xts_by_chunk[c]
            ph = ppool.tile([128, CH], F32, tag="psum", name="ph")
            for k in range(KT):
                nc.tensor.matmul(
                    ph[:],
                    lhsT=wts[JT + j][:, k * 128:(k + 1) * 128],
                    rhs=xl[k](0, CH),
                    start=(k == 0),
                    stop=(k == KT - 1),
                )
            tiles = []
            for f0 in range(0, CH, sub):
                fs = slice(f0, f0 + sub)
                sg_t = spool.tile([128, sub], F32, tag="sg")
                nc.scalar.activation(sg_t[:], ph[:, fs], AF.Sigmoid,
                                     bias=bias_t[:, JT + j:JT + j + 1],
                                     scale=1.0)
                r_t = spool.tile([128, sub], F32, tag="r")
                nc.scalar.activation(r_t[:], ph[:, fs], AF.Relu,
                                     bias=bias_t[:, JT + j:JT + j + 1],
                                     scale=1.0)
                # g = min(sigmoid(hidden), 0.5) + relu(hidden)
                if pool is None:
                    g_t = spool.tile([128, sub], F32, tag="g")
                else:
                    g_t = pool.tile([128, sub], F32, tag=f"g7_{f0}")
                nc.vector.scalar_tensor_tensor(g_t[:], sg_t[:], 0.5,
                                               r_t[:], op0=OP.min,
                                               op1=OP.add)
                tiles.append((f0, sub, g_t))

            def g_at(f0, w):
                for t0, tw, t in tiles:
                    if t0 <= f0 and f0 + w <= t0 + tw:
                        return t[:, f0 - t0:f0 - t0 + w]
                raise KeyError((f0, w))
            return g_at

        def gate_part(c, j, sub, g_at, gate_split=1, a_on_dve=False):
            """Gate-half matmuls + a/z/b/scan/store for chain (c,j).
            `gate_split` > 1 runs the gate matmuls as that many column
            sub-chains so the earlier blocks' post-ops overlap the later
            blocks' matmuls, and `a_on_dve` computes a = 1-z on the DVE
            instead of a second ACT sigmoid (both shorten the serial
            post-matmul tail; used for the very last chain only)."""
            xl = xts_by_chunk[c]
            s0 = c * CH
            pg = ppool.tile([128, CH], F32, tag="psum", name="pg")
            gw = CH // gate_split
            for lo in range(0, CH, gw):
                for k in range(KT):
                    nc.tensor.matmul(
                        pg[:, lo:lo + gw],
                        lhsT=wts[j][:, k * 128:(k + 1) * 128],
                        rhs=xl[k](lo, lo + gw),
                        start=(k == 0),
                        stop=(k == KT - 1),
                    )
            h_t = hpool.tile([128, CH], F32, tag=f"h{j}")
            for f0 in range(0, CH, sub):
                fs = slice(f0, f0 + sub)
                z_t = spool.tile([128, sub], F32, tag="z")
                nc.scalar.activation(z_t[:], pg[:, fs], AF.Sigmoid,
                                     bias=bias_t[:, j:j + 1], scale=1.0)
                a_t = abpool.tile([128, sub], F32, tag="a")
                if a_on_dve:
                    nc.vector.tensor_scalar(a_t[:], z_t[:], -1.0, 1.0,
                                            op0=OP.mult, op1=OP.add)
                else:
                    nc.scalar.activation(a_t[:], pg[:, fs], AF.Sigmoid,
                                         bias=nbias_t[:, j:j + 1], scale=-1.0)
                b_t = abpool.tile([128, sub], F32, tag="b")
                nc.vector.tensor_mul(b_t[:], z_t[:], g_at(f0, sub))
                # ---- scan: h = a*h_prev + b along time
                if f0 == 0:
                    init = (h0_t[:, j:j + 1] if c == 0
                            else prev_h[j][:, CH - 1:CH])
                else:
                    init = h_t[:, f0 - 1:f0]
                nc.vector.tensor_tensor_scan(h_t[:, fs], a_t[:], b_t[:],
                                             init, op0=OP.mult, op1=OP.add)
            prev_h[j] = h_t
            # GpSimd's end-of-kernel DRAIN detects DMA completion
            # slowly (~6 us); keep the final chunks' stores on Sync
            # (idle by then) so the kernel end isn't gated on it.
            # The final chunk stores per-sub so the last transfer
            # is short.
            out_q = nc.gpsimd if c < NCHUNK - 2 else nc.sync
            if c == NCHUNK - 1:
                for f0 in range(0, CH, sub):
                    out_q.dma_start(
                        out[j * 128:(j + 1) * 128, s0 + f0:s0 + f0 + sub],
                        h_t[:, f0:f0 + sub])
            else:
                out_q.dma_start(out[j * 128:(j + 1) * 128, s0:s0 + CH],
                                h_t[:])

        def chain(c, j, sub, gate_split=1, a_on_dve=False):
            gate_part(c, j, sub, hidden_part(c, j, sub), gate_split,
                      a_on_dve)

        # Chunks 0/1: single-chunk chains, interleaved so chains line
        # up with the W/x arrival schedule.
        for c, j in ([(0, j) for j in range(6)] + [(1, j) for j in range(4)]
                     + [(0, 6), (0, 7)] + [(1, j) for j in range(4, 8)]):
            chain(c, j, CH)
        # Chunks 2..7 streamed singly with batched x transfers.  The last
        # chain's hidden half is hoisted to the top of the final chunk so
        # the end-of-kernel tail is only its gate post-ops; those run at
        # 128 wide over two gate matmul half-chains so the serial
        # a/z/mul/scan/store tail after the last matmul is short.
        for c in range(2, NCHUNK):
            x_load_big(c)
            if c == NCHUNK - 1:
                for j in range(JT - 1):
                    chain(c, j, CH if j < JT - 2 else CH // 2)
                chain(c, JT - 1, CH // 2, gate_split=2, a_on_dve=True)
            else:
                for j in range(JT):
                    chain(c, j, CH)

    nc.compile()
    return nc


def _run(inputs, trace=False, **spmd_kwargs):
    x = np.asarray(inputs["x"], dtype=np.float32)
    h = np.asarray(inputs["h"], dtype=np.float32)
    W = np.asarray(inputs["W"], dtype=np.float32)
    b = np.asarray(inputs["b"], dtype=np.float32)

    xt_all = np.ascontiguousarray(x.transpose(0, 2, 1)).astype(np.float16).reshape(
        B, KT, 128, S)                                             # (B,KT,128,S)
    # wp[o*128+p, k*128+c] = W[o*128+c, k*128+p]
    WP = np.ascontiguousarray(
        W.reshape(2 * JT, 128, KT, 128).transpose(0, 3, 2, 1)
        .reshape(2 * DH, DIN)).astype(np.float16)
    bias_t = np.ascontiguousarray(b.reshape(2 * JT, 128).T)        # (128, 2JT)
    nbias_t = np.ascontiguousarray(-bias_t)
    h0_all = np.ascontiguousarray(
        h[:, 0, :].reshape(B, JT, 128).transpose(0, 2, 1))         # (B, 128, JT)

    if "prog" not in _prog_cache:
        _prog_cache["prog"] = _build_program()
    nc = _prog_cache["prog"]

    in_maps = [
        {"xt": xt_all[c], "wp": WP, "bias": bias_t, "nbias": nbias_t,
         "h0": h0_all[c]}
        for c in range(B)
    ]
    res = run_bass_kernel_spmd(nc, in_maps, list(range(B)), trace=trace,
                               **spmd_kwargs)
    out = np.stack([res.results[c]["out"].T for c in range(B)], axis=0)
    return np.ascontiguousarray(out), res


def kernel(**inputs) -> np.ndarray:
    return _run(inputs)[0]

